# revision 7
# baseline (speedup 1.0000x reference)
"""Trainium2 Bass kernel for nn_BinaryConv2d (B=16, C=64, H=W=256, 3x3, pad 1).

Forward semantics (STE forward values):
  act = sign(x * rd_k + rd_b)                  in {-1, 0, +1}
  bw  = scaling[co] * sign(conv_w)             scaling = mean |conv_w| per out-ch
  y   = conv2d(act, bw, pad=1)
  y   = prelu(y + pr_bias0) + pr_bias1 + x     prelu slope per channel

Strategy: data-parallel over batch, 2 images per core (8 cores).  The two
images' 64 channels are stacked on the 128 SBUF partitions.  x is shipped as
bf16 (residual-precision is ample for the 2e-2 gate) and y is returned as
bf16, halving HBM traffic.  Activations are binarized to fp8 +-1 on the
Scalar engine; the 3x3 conv is 5 PSUM matmuls per output row with
block-diagonal +-1 fp8 weights (exact integer arithmetic in fp32 PSUM):
3 DoubleRow matmuls pair kh=0/1 per kw, a 4th DoubleRow pairs the kh=2
kw=0/1 taps via an overlapping stride-1 rhs AP, and kh=2,kw=2 is a plain
matmul.  Post-ops: the PSUM drain (fused scale+bias0, bf16 out) alternates
between ACT and DVE; DVE runs the PReLU min/mult in bf16 4x/2x modes and
Pool does the residual add.
"""

import sys

if "/opt/trn_rl_repo" not in sys.path:
    sys.path.insert(0, "/opt/trn_rl_repo")

from contextlib import ExitStack

import ml_dtypes
import numpy as np

import concourse.bacc as bacc
import concourse.bass as bass
import concourse.tile as tile
from concourse import mybir
from concourse.ap import AP
from concourse.bass_utils import run_bass_kernel_spmd

B, C, H, W = 16, 64, 256, 256
NCORES = 8
P = 128                      # partitions = 2 images x 64 channels

F32 = mybir.dt.float32
BF16 = mybir.dt.bfloat16
FP8 = mybir.dt.float8e4
AF = mybir.ActivationFunctionType
ALU = mybir.AluOpType
DR = mybir.MatmulPerfMode.DoubleRow

APITCH = 272                 # act row pitch (bytes %16 for DoubleRow AP steps)

# Param table columns (per-partition f32 scalars)
PK, PB, PS, PB0, PCM = 0, 1, 2, 3, 4

# kh=2 row handled as DoubleRow over (kw=0, kw=1) with an overlapping
# stride-1 rhs AP + one plain kw=2 matmul (10W streamed columns per 2-row
# tile).  False falls back to one plain 2-row matmul per kw (12W).
KH2_DR = True
# The GPSIMD/Pool engine cannot read PSUM (walrus verifier), so the
# v = ps*scale + bias0 drain alternates between ACT (Identity activation)
# and DVE (tensor_scalar) per 2-row tile; 'act' puts it all on ACT.
DRAIN_ENGINE = "split"

SIGN_CHUNK = 9               # rows of sign-activation per ACT instruction
STRIP_HS = [32] * 8          # strip heights (sum == H)


def _emit(tc, nc, x_d, w_d, p_d, y_d):
    x3 = x_d.rearrange("p (h w) -> p h w", w=W)
    y3 = y_d.rearrange("p (h w) -> p h w", w=W)

    with ExitStack() as ctx:
        consts = ctx.enter_context(tc.tile_pool(name="consts", bufs=1))
        xpool = ctx.enter_context(tc.tile_pool(name="xpool", bufs=2))
        apool = ctx.enter_context(tc.tile_pool(name="apool", bufs=2))
        ypool = ctx.enter_context(tc.tile_pool(name="ypool", bufs=2))
        vpool = ctx.enter_context(tc.tile_pool(name="vpool", bufs=4))
        mpool = ctx.enter_context(tc.tile_pool(name="mpool", bufs=4))
        upool = ctx.enter_context(tc.tile_pool(name="upool", bufs=4))
        pspool = ctx.enter_context(tc.tile_pool(name="pspool", bufs=8,
                                                space="PSUM"))

        # params first on the load ring (sign needs them); weights on the
        # store ring, which is idle at kernel start
        pt = consts.tile([P, 8], F32)
        nc.sync.dma_start(out=pt, in_=p_d)
        # [kw, delta(kh 0/1), m] DoubleRow weights
        wdr = consts.tile([P, 3, 2, 128], FP8)
        nc.scalar.dma_start(out=wdr, in_=w_d[:, :768].rearrange(
            "p (k d m) -> p k d m", k=3, d=2))
        # kh=2: [delta(kw 0/1), m] DoubleRow + kw=2 plain
        wk2 = consts.tile([P, 2, 128], FP8)
        nc.scalar.dma_start(out=wk2, in_=w_d[:, 768:1024].rearrange(
            "p (d m) -> p d m", d=2))
        wn2 = consts.tile([P, 128], FP8)
        nc.scalar.dma_start(out=wn2, in_=w_d[:, 1024:])

        H0S = [sum(STRIP_HS[:i]) for i in range(len(STRIP_HS))]
        NST = len(STRIP_HS)
        HSMAX = max(STRIP_HS)

        def strip_rows(s):
            h0 = H0S[s]
            row_lo = max(h0 - 1, 0)
            row_hi = min(h0 + STRIP_HS[s] + 1, H)
            return h0, row_lo, row_hi, row_lo - (h0 - 1)

        def load_strip(s):
            """DMA the x strip (rows h0-1 .. h0+hs; tile row a <-> global
            h0-1+a) and memset the act padding."""
            h0, row_lo, row_hi, r0 = strip_rows(s)
            nr = row_hi - row_lo
            xs = xpool.tile([P, HSMAX + 2, W], BF16, name="xs")
            nld = 4 if s == 0 else 2     # strip 0 in quarters: faster start
            bounds = [row_lo + (nr * i) // nld for i in range(nld + 1)]
            for a, b in zip(bounds, bounds[1:]):
                if b > a:
                    nc.sync.dma_start(out=xs[:, a - (h0 - 1):b - (h0 - 1), :],
                                      in_=x3[:, a:b, :])
            act = apool.tile([P, HSMAX + 2, APITCH], FP8, name="act")
            nrows = STRIP_HS[s] + 2
            nc.gpsimd.memset(act[:, :nrows, 0:1], 0.0)
            nc.gpsimd.memset(act[:, :nrows, W + 1:W + 2], 0.0)
            if s == 0:
                nc.gpsimd.memset(act[:, 0:1, :], 0.0)
            if s == NST - 1:
                nc.gpsimd.memset(act[:, nrows - 1:nrows, :], 0.0)
            return xs, act

        def sign_strip(s, xs, act, chunks, skip=0):
            """Binarize x into the zero-padded act tile, in row chunks (the
            first small so dependent matmuls unblock quickly)."""
            _, row_lo, row_hi, r0 = strip_rows(s)
            c0 = r0 + skip
            for sz in chunks:
                c1 = min(c0 + sz, r0 + (row_hi - row_lo))
                if c1 <= c0:
                    break
                nc.scalar.activation(
                    act[:, c0:c1, 1:W + 1], xs[:, c0:c1, :], AF.Sign,
                    bias=pt[:, PB:PB + 1], scale=pt[:, PK:PK + 1],
                )
                c0 = c1

        def kh2_overlap_rhs(act, row):
            """[P, 2, W] rhs with both free strides 1: element (d, m) reads
            padded act col d+m of `row`, pairing the kh=2 kw=0/1 taps."""
            base = act[:, row, 0:W]
            return AP(base.tensor, base.offset,
                      [list(base.ap[0]), [1, 2], [1, W]])

        FIRST_CHUNKS = (5,) * 7 + (4,)   # strip 0: progressive chunks
        NEXT_CHUNKS = (5,) + (SIGN_CHUNK,) * 4

        SPLIT_FIRST_CHUNK = True
        cur = load_strip(0)
        sign_strip(0, *cur, FIRST_CHUNKS)
        nxt = None
        for s in range(NST):
            h0 = H0S[s]
            HS_S = STRIP_HS[s]
            MT = HS_S // 2
            xs, act = cur
            ys = ypool.tile([P, HSMAX, W], BF16, name="ys")
            for mt in range(MT):
                if mt == min(1, MT - 1) and s + 1 < NST and SPLIT_FIRST_CHUNK:
                    nxt = load_strip(s + 1)   # loads overlap this strip
                if mt == max(MT - 2, 0) and s + 1 < NST and SPLIT_FIRST_CHUNK:
                    # data definitely landed; ACT binarizes it while the PE
                    # finishes this strip
                    sign_strip(s + 1, *nxt, NEXT_CHUNKS[:1])
                ps = pspool.tile([P, 2, W], F32, name="ps")
                for i in range(2):
                    r = 2 * mt + i       # act tile row of the first tap
                    po = ps[:, i, :]
                    for kw in range(3):
                        # kh in {0,1} via DoubleRow: contraction over
                        # (partition, delta), act row r+delta
                        nc.tensor.matmul(
                            po, lhsT=wdr[:, kw, :, :],
                            rhs=act[:, r:r + 2, kw:kw + W],
                            start=(kw == 0 and i == 0), stop=False,
                            perf_mode=DR,
                        )
                    if KH2_DR:
                        # kh=2, kw in {0,1} via DoubleRow on overlapping
                        # stride-1 columns of act row r+2
                        nc.tensor.matmul(
                            po, lhsT=wk2,
                            rhs=kh2_overlap_rhs(act, r + 2),
                            start=False, stop=False, perf_mode=DR,
                        )
                        nc.tensor.matmul(
                            po, lhsT=wn2, rhs=act[:, r + 2, 2:2 + W],
                            start=False, stop=(i == 1),
                        )
                if not KH2_DR:
                    for kw in range(3):
                        nc.tensor.matmul(
                            ps, lhsT=wk2[:, 0, :] if kw == 0 else
                            (wk2[:, 1, :] if kw == 1 else wn2),
                            rhs=act[:, 2 * mt + 2:2 * mt + 4, kw:kw + W],
                            start=False, stop=(kw == 2),
                        )
                # v = ps*scaling + b0 (bf16): per 2-row psum tile; prelu and
                # residual run at 4-row granularity in DVE 2x/4x modes
                if mt % 2 == 0:
                    v4 = vpool.tile([P, 4, W], BF16, name="v")
                half = (mt % 2) * 2
                vslice = v4[:, half:half + 2, :]
                if DRAIN_ENGINE == "split" and mt % 2 == 1:
                    nc.vector.tensor_scalar(
                        vslice, ps, pt[:, PS:PS + 1], pt[:, PB0:PB0 + 1],
                        ALU.mult, ALU.add,
                    )
                else:
                    nc.scalar.activation(
                        vslice, ps, AF.Identity,
                        bias=pt[:, PB0:PB0 + 1], scale=pt[:, PS:PS + 1],
                    )
                if mt % 2 == 1:
                    r4 = 2 * (mt - 1)
                    m4 = mpool.tile([P, 4, W], BF16, name="m")
                    u4 = upool.tile([P, 4, W], BF16, name="u")
                    x4 = xs[:, r4 + 1:r4 + 5, :]
                    # prelu(v) = v + (slope-1)*min(v, 0); bf16 ts runs 4x
                    nc.vector.tensor_scalar(
                        m4, v4, 0.0, pt[:, PCM:PCM + 1], ALU.min, ALU.mult,
                    )
                    nc.vector.tensor_tensor(u4, v4, m4, ALU.add)
                    # residual (+ pr_bias1, folded into x on the host) on
                    # Pool, which is otherwise idle
                    nc.gpsimd.tensor_tensor(
                        ys[:, r4:r4 + 4, :], u4, x4, ALU.add,
                    )
            # stores on the ACT HWDGE ring (separate queue from loads)
            nq = 2 if HS_S > 8 else 1
            for q in range(nq):
                r = q * (HS_S // nq)
                r1 = (q + 1) * (HS_S // nq)
                nc.scalar.dma_start(out=y3[:, h0 + r:h0 + r1, :],
                                    in_=ys[:, r:r1, :])
            if s + 1 < NST:
                if SPLIT_FIRST_CHUNK:
                    sign_strip(s + 1, *nxt, NEXT_CHUNKS[1:],
                               skip=NEXT_CHUNKS[0])
                else:
                    nxt = load_strip(s + 1)
                    sign_strip(s + 1, *nxt, NEXT_CHUNKS)
            cur = nxt


def build_nc():
    nc = bacc.Bacc("TRN2", target_bir_lowering=False, debug=False,
                   num_devices=NCORES)
    x_d = nc.dram_tensor("xin", [P, H * W], BF16, kind="ExternalInput").ap()
    w_d = nc.dram_tensor("wp", [P, 9 * 128], FP8, kind="ExternalInput").ap()
    p_d = nc.dram_tensor("pp", [P, 8], F32, kind="ExternalInput").ap()
    y_d = nc.dram_tensor("yout", [P, H * W], BF16, kind="ExternalOutput").ap()
    with tile.TileContext(nc) as tc:
        _emit(tc, nc, x_d, w_d, p_d, y_d)
    nc.compile()
    return nc


_NC_CACHE = {}


def _get_nc():
    key = (KH2_DR, DRAIN_ENGINE)
    if key not in _NC_CACHE:
        _NC_CACHE[key] = build_nc()
    return _NC_CACHE[key]


def make_inputs(x, rd_k, rd_b, beta, conv_w, pr_bias0, prelu_w, pr_bias1):
    """Host-side prep: per-channel param table, packed sign weights, shards."""
    k = np.asarray(rd_k, np.float32).reshape(C)
    b = np.asarray(rd_b, np.float32).reshape(C)
    s = np.mean(np.abs(np.asarray(conv_w, np.float32)), axis=(1, 2, 3))
    b0 = np.asarray(pr_bias0, np.float32).reshape(C)
    slope = np.asarray(prelu_w, np.float32).reshape(C)
    b1 = np.asarray(pr_bias1, np.float32).reshape(C)
    cm = slope - 1.0
    # pr_bias1 is folded into the residual input x' = x + b1; the sign
    # threshold compensates: sign(k*x + b) == sign(k*x' + (b - k*b1))
    badj = b - k * b1
    cols = np.stack([k, badj, s, b0, cm,
                     np.zeros(C, np.float32), np.zeros(C, np.float32),
                     np.zeros(C, np.float32)], axis=1)
    pp = np.concatenate([cols, cols], axis=0).astype(np.float32)  # [128, 8]

    sw = np.sign(np.asarray(conv_w, np.float32)).astype(np.float32)

    def blockdiag(kh, kw):
        S = sw[:, :, kh, kw].T  # [ci, co]
        out = np.zeros((P, P), np.float32)
        out[0:C, 0:C] = S
        out[C:P, C:P] = S
        return out

    wp = np.zeros((P, 9, 128), np.float32)
    for kw in range(3):            # [kw, delta(kh 0/1), m] DoubleRow pairs
        for d in range(2):
            wp[:, kw * 2 + d, :] = blockdiag(d, kw)
    for d in range(2):             # kh=2: [delta(kw 0/1), m] DoubleRow
        wp[:, 6 + d, :] = blockdiag(2, d)
    wp[:, 8, :] = blockdiag(2, 2)  # kh=2, kw=2 plain
    wp = np.ascontiguousarray(wp.reshape(P, 9 * 128)).astype(
        mybir.dt.np(FP8))

    xr = np.asarray(x, np.float32) + b1[None, :, None, None]
    xr = xr.astype(ml_dtypes.bfloat16)
    in_maps = []
    for c in range(NCORES):
        xc = np.ascontiguousarray(xr[2 * c:2 * c + 2]).reshape(P, H * W)
        in_maps.append({"xin": xc, "wp": wp, "pp": pp})
    return in_maps


def kernel(x, rd_k, rd_b, beta, conv_w, pr_bias0, prelu_w, pr_bias1):
    in_maps = make_inputs(x, rd_k, rd_b, beta, conv_w, pr_bias0, prelu_w,
                          pr_bias1)
    nc = _get_nc()
    res = run_bass_kernel_spmd(nc, in_maps, core_ids=list(range(NCORES)))
    y = np.empty((B, C, H, W), np.float32)
    for c in range(NCORES):
        y[2 * c:2 * c + 2] = np.asarray(
            res.results[c]["yout"]).astype(np.float32).reshape(2, C, H, W)
    return y


# revision 18
# speedup vs baseline: 1.0414x; 1.0414x over previous
"""Trainium2 Bass kernel for nn_BinaryConv2d (B=16, C=64, H=W=256, 3x3, pad 1).

Forward semantics (STE forward values):
  act = sign(x * rd_k + rd_b)                  in {-1, 0, +1}
  bw  = scaling[co] * sign(conv_w)             scaling = mean |conv_w| per out-ch
  y   = conv2d(act, bw, pad=1)
  y   = prelu(y + pr_bias0) + pr_bias1 + x     prelu slope per channel

Strategy: data-parallel over batch, 2 images per core (8 cores).  The two
images' 64 channels are stacked on the 128 SBUF partitions.  x is shipped as
bf16 (residual-precision is ample for the 2e-2 gate) and y is returned as
bf16, halving HBM traffic.  Activations are binarized to fp8 +-1 on the
Scalar engine; the 3x3 conv is 5 PSUM matmuls per output row with
block-diagonal +-1 fp8 weights (exact integer arithmetic in fp32 PSUM):
3 DoubleRow matmuls pair kh=0/1 per kw, a 4th DoubleRow pairs the kh=2
kw=0/1 taps via an overlapping stride-1 rhs AP, and kh=2,kw=2 is a plain
matmul.  Post-ops: the PSUM drain (fused scale+bias0, bf16 out) alternates
between ACT and DVE; DVE runs the PReLU min/mult in bf16 4x/2x modes and
Pool does the residual add.
"""

import sys

if "/opt/trn_rl_repo" not in sys.path:
    sys.path.insert(0, "/opt/trn_rl_repo")

from contextlib import ExitStack

import ml_dtypes
import numpy as np

import concourse.bacc as bacc
import concourse.bass as bass
import concourse.tile as tile
from concourse import mybir
from concourse.ap import AP
from concourse.bass_utils import run_bass_kernel_spmd

B, C, H, W = 16, 64, 256, 256
NCORES = 8
P = 128                      # partitions = 2 images x 64 channels

F32 = mybir.dt.float32
BF16 = mybir.dt.bfloat16
FP8 = mybir.dt.float8e4
AF = mybir.ActivationFunctionType
ALU = mybir.AluOpType
DR = mybir.MatmulPerfMode.DoubleRow

APITCH = 272                 # act row pitch (bytes %16 for DoubleRow AP steps)

# Param table columns (per-partition f32 scalars)
PK, PB, PS, PB0, PCM, PSL, PSAM, PZ = range(8)

# kh=2 row handled as DoubleRow over (kw=0, kw=1) with an overlapping
# stride-1 rhs AP + one plain kw=2 matmul (10W streamed columns per 2-row
# tile).  False falls back to one plain 2-row matmul per kw (12W).
KH2_DR = True
# Fast path requires pr_bias0 == 0 and 0 <= slope <= 1 (true for the conv
# init this model ships with); kernel() falls back to the general path
# otherwise.  Measured-cost notes: GPSIMD cannot read PSUM; Lrelu is broken
# on TRN2 hw; DVE bf16 tensor_scalar runs ~2.5x, stt has no accel but is a
# stable ~1.2ns/elem; Pool tensor ops run at ~2.2ns/elem.
FAST_POST = True
# fraction of 4-row groups whose PSUM drain+prelu runs on ACT (Identity)
# + DVE (stt max); the rest run entirely on DVE via the relu-ts m-path
# (b0=0 only).  Balances ACT (sign-heavy) against DVE.
ACT_DRAIN_NUM, ACT_DRAIN_DEN = 4, 5

SIGN_CHUNK = 9               # rows of sign-activation per ACT instruction
STRIP_HS = [32] * 8          # strip heights (sum == H)


def _emit(tc, nc, x_d, w_d, p_d, y_d):
    x3 = x_d.rearrange("p (h w) -> p h w", w=W)
    y3 = y_d.rearrange("p (h w) -> p h w", w=W)

    with ExitStack() as ctx:
        consts = ctx.enter_context(tc.tile_pool(name="consts", bufs=1))
        xpool = ctx.enter_context(tc.tile_pool(name="xpool", bufs=2))
        apool = ctx.enter_context(tc.tile_pool(name="apool", bufs=2))
        ypool = ctx.enter_context(tc.tile_pool(name="ypool", bufs=2))
        vpool = ctx.enter_context(tc.tile_pool(name="vpool", bufs=4))
        mpool = ctx.enter_context(tc.tile_pool(name="mpool", bufs=4))
        upool = ctx.enter_context(tc.tile_pool(name="upool", bufs=4))
        pspool = ctx.enter_context(tc.tile_pool(name="pspool", bufs=4,
                                                space="PSUM"))

        # params first on the load ring (sign needs them); weights on the
        # store ring, which is idle at kernel start
        pt = consts.tile([P, 8], F32)
        nc.sync.dma_start(out=pt, in_=p_d)
        # [kw, delta(kh 0/1), m] DoubleRow weights
        wdr = consts.tile([P, 3, 2, 128], FP8)
        nc.scalar.dma_start(out=wdr, in_=w_d[:, :768].rearrange(
            "p (k d m) -> p k d m", k=3, d=2))
        # kh=2: [delta(kw 0/1), m] DoubleRow + kw=2 plain
        wk2 = consts.tile([P, 2, 128], FP8)
        nc.scalar.dma_start(out=wk2, in_=w_d[:, 768:1024].rearrange(
            "p (d m) -> p d m", d=2))
        wn2 = consts.tile([P, 128], FP8)
        nc.scalar.dma_start(out=wn2, in_=w_d[:, 1024:])

        H0S = [sum(STRIP_HS[:i]) for i in range(len(STRIP_HS))]
        NST = len(STRIP_HS)
        HSMAX = max(STRIP_HS)

        def strip_rows(s):
            h0 = H0S[s]
            row_lo = max(h0 - 1, 0)
            row_hi = min(h0 + STRIP_HS[s] + 1, H)
            return h0, row_lo, row_hi, row_lo - (h0 - 1)

        def load_strip(s):
            """DMA the x strip (rows h0-1 .. h0+hs; tile row a <-> global
            h0-1+a) and memset the act padding."""
            h0, row_lo, row_hi, r0 = strip_rows(s)
            nr = row_hi - row_lo
            xs = xpool.tile([P, HSMAX + 2, W], BF16, name="xs")
            nld = 4 if s == 0 else 2     # strip 0 in quarters: faster start
            bounds = [row_lo + (nr * i) // nld for i in range(nld + 1)]
            for a, b in zip(bounds, bounds[1:]):
                if b > a:
                    nc.sync.dma_start(out=xs[:, a - (h0 - 1):b - (h0 - 1), :],
                                      in_=x3[:, a:b, :])
            act = apool.tile([P, HSMAX + 2, APITCH], FP8, name="act")
            nrows = STRIP_HS[s] + 2
            nc.gpsimd.memset(act[:, :nrows, 0:1], 0.0)
            nc.gpsimd.memset(act[:, :nrows, W + 1:W + 2], 0.0)
            if s == 0:
                nc.gpsimd.memset(act[:, 0:1, :], 0.0)
            if s == NST - 1:
                nc.gpsimd.memset(act[:, nrows - 1:nrows, :], 0.0)
            return xs, act

        def sign_strip(s, xs, act, chunks, skip=0):
            """Binarize x into the zero-padded act tile, in row chunks (the
            first small so dependent matmuls unblock quickly)."""
            _, row_lo, row_hi, r0 = strip_rows(s)
            c0 = r0 + skip
            for sz in chunks:
                c1 = min(c0 + sz, r0 + (row_hi - row_lo))
                if c1 <= c0:
                    break
                nc.scalar.activation(
                    act[:, c0:c1, 1:W + 1], xs[:, c0:c1, :], AF.Sign,
                    bias=pt[:, PB:PB + 1], scale=pt[:, PK:PK + 1],
                )
                c0 = c1

        def kh2_overlap_rhs(act, row):
            """[P, 2, W] rhs with both free strides 1: element (d, m) reads
            padded act col d+m of `row`, pairing the kh=2 kw=0/1 taps."""
            base = act[:, row, 0:W]
            return AP(base.tensor, base.offset,
                      [list(base.ap[0]), [1, 2], [1, W]])

        FIRST_CHUNKS = (5,) * 7 + (4,)   # strip 0: progressive chunks
        NEXT_CHUNKS = (5,) + (SIGN_CHUNK,) * 4

        SPLIT_FIRST_CHUNK = True
        cur = load_strip(0)
        sign_strip(0, *cur, FIRST_CHUNKS)
        nxt = None
        for s in range(NST):
            h0 = H0S[s]
            HS_S = STRIP_HS[s]
            MT = HS_S // 2
            xs, act = cur
            ys = ypool.tile([P, HSMAX, W], BF16, name="ys")
            for mt in range(MT):
                if mt == min(1, MT - 1) and s + 1 < NST and SPLIT_FIRST_CHUNK:
                    nxt = load_strip(s + 1)   # loads overlap this strip
                if mt == max(MT - 2, 0) and s + 1 < NST and SPLIT_FIRST_CHUNK:
                    # data definitely landed; ACT binarizes it while the PE
                    # finishes this strip
                    sign_strip(s + 1, *nxt, NEXT_CHUNKS[:1])
                if mt % 2 == 0:
                    # 4-row PSUM tile spanning 2 banks; each mt fills one
                    # bank (start/stop delimit per-bank accumulation groups)
                    ps4 = pspool.tile([P, 4, W], F32, name="ps")
                for i in range(2):
                    r = 2 * mt + i       # act tile row of the first tap
                    po = ps4[:, 2 * (mt % 2) + i, :]
                    for kw in range(3):
                        # kh in {0,1} via DoubleRow: contraction over
                        # (partition, delta), act row r+delta
                        nc.tensor.matmul(
                            po, lhsT=wdr[:, kw, :, :],
                            rhs=act[:, r:r + 2, kw:kw + W],
                            start=(kw == 0 and i == 0), stop=False,
                            perf_mode=DR,
                        )
                    if KH2_DR:
                        # kh=2, kw in {0,1} via DoubleRow on overlapping
                        # stride-1 columns of act row r+2
                        nc.tensor.matmul(
                            po, lhsT=wk2,
                            rhs=kh2_overlap_rhs(act, r + 2),
                            start=False, stop=False, perf_mode=DR,
                        )
                        nc.tensor.matmul(
                            po, lhsT=wn2, rhs=act[:, r + 2, 2:2 + W],
                            start=False, stop=(i == 1),
                        )
                if not KH2_DR:
                    for kw in range(3):
                        nc.tensor.matmul(
                            ps4[:, 2 * (mt % 2):2 * (mt % 2) + 2, :],
                            lhsT=wk2[:, 0, :] if kw == 0 else
                            (wk2[:, 1, :] if kw == 1 else wn2),
                            rhs=act[:, 2 * mt + 2:2 * mt + 4, kw:kw + W],
                            start=False, stop=(kw == 2),
                        )
                if mt % 2 == 1:
                    g = (s * MT + mt) // 2
                    r4 = 2 * (mt - 1)
                    u4 = upool.tile([P, 4, W], BF16, name="u")
                    x4 = xs[:, r4 + 1:r4 + 5, :]
                    y4 = ys[:, r4:r4 + 4, :]
                    if FAST_POST and g % ACT_DRAIN_DEN < ACT_DRAIN_NUM:
                        # ACT drains (v = s*ps + b0, bf16), DVE does the
                        # prelu as one stt: u = max(v, slope*v)
                        v4 = vpool.tile([P, 4, W], BF16, name="v")
                        nc.scalar.activation(
                            v4, ps4, AF.Identity,
                            bias=pt[:, PB0:PB0 + 1], scale=pt[:, PS:PS + 1],
                        )
                        nc.vector.scalar_tensor_tensor(
                            u4, v4, pt[:, PSL:PSL + 1], v4, ALU.mult, ALU.max,
                        )
                    elif FAST_POST:
                        # all-DVE m-path (valid for b0 == 0):
                        # m = relu(s*(slope-1)*ps) = s*(slope-1)*min(ps, 0)
                        # u = s*ps + m = s * prelu(ps)
                        m4 = mpool.tile([P, 4, W], BF16, name="m")
                        nc.vector.tensor_scalar(
                            m4, ps4, pt[:, PSAM:PSAM + 1], 0.0,
                            ALU.mult, ALU.max,
                        )
                        nc.vector.scalar_tensor_tensor(
                            u4, ps4, pt[:, PS:PS + 1], m4, ALU.mult, ALU.add,
                        )
                    else:
                        # general path: ACT drain, DVE min/mult + add
                        v4 = vpool.tile([P, 4, W], BF16, name="v")
                        m4 = mpool.tile([P, 4, W], BF16, name="m")
                        nc.scalar.activation(
                            v4, ps4, AF.Identity,
                            bias=pt[:, PB0:PB0 + 1], scale=pt[:, PS:PS + 1],
                        )
                        nc.vector.tensor_scalar(
                            m4, v4, 0.0, pt[:, PCM:PCM + 1], ALU.min, ALU.mult,
                        )
                        nc.vector.tensor_tensor(u4, v4, m4, ALU.add)
                    # residual (+ pr_bias1, folded into x on the host),
                    # alternating DVE / Pool to balance load
                    if g % 2 == 0:
                        nc.vector.scalar_tensor_tensor(
                            y4, x4, pt[:, PZ:PZ + 1], u4, ALU.add, ALU.add,
                        )
                    else:
                        nc.gpsimd.tensor_tensor(y4, u4, x4, ALU.add)
            # stores on the ACT HWDGE ring (separate queue from loads)
            nq = 2 if HS_S > 8 else 1
            for q in range(nq):
                r = q * (HS_S // nq)
                r1 = (q + 1) * (HS_S // nq)
                nc.scalar.dma_start(out=y3[:, h0 + r:h0 + r1, :],
                                    in_=ys[:, r:r1, :])
            if s + 1 < NST:
                if SPLIT_FIRST_CHUNK:
                    sign_strip(s + 1, *nxt, NEXT_CHUNKS[1:],
                               skip=NEXT_CHUNKS[0])
                else:
                    nxt = load_strip(s + 1)
                    sign_strip(s + 1, *nxt, NEXT_CHUNKS)
            cur = nxt


def build_nc():
    nc = bacc.Bacc("TRN2", target_bir_lowering=False, debug=False,
                   num_devices=NCORES)
    x_d = nc.dram_tensor("xin", [P, H * W], BF16, kind="ExternalInput").ap()
    w_d = nc.dram_tensor("wp", [P, 9 * 128], FP8, kind="ExternalInput").ap()
    p_d = nc.dram_tensor("pp", [P, 8], F32, kind="ExternalInput").ap()
    y_d = nc.dram_tensor("yout", [P, H * W], BF16, kind="ExternalOutput").ap()
    with tile.TileContext(nc) as tc:
        _emit(tc, nc, x_d, w_d, p_d, y_d)
    nc.compile()
    return nc


_NC_CACHE = {}


def _get_nc():
    key = (KH2_DR, FAST_POST, ACT_DRAIN_NUM, ACT_DRAIN_DEN)
    if key not in _NC_CACHE:
        _NC_CACHE[key] = build_nc()
    return _NC_CACHE[key]


def make_inputs(x, rd_k, rd_b, beta, conv_w, pr_bias0, prelu_w, pr_bias1):
    """Host-side prep: per-channel param table, packed sign weights, shards."""
    k = np.asarray(rd_k, np.float32).reshape(C)
    b = np.asarray(rd_b, np.float32).reshape(C)
    s = np.mean(np.abs(np.asarray(conv_w, np.float32)), axis=(1, 2, 3))
    b0 = np.asarray(pr_bias0, np.float32).reshape(C)
    slope = np.asarray(prelu_w, np.float32).reshape(C)
    b1 = np.asarray(pr_bias1, np.float32).reshape(C)
    cm = slope - 1.0
    # pr_bias1 is folded into the residual input x' = x + b1; the sign
    # threshold compensates: sign(k*x + b) == sign(k*x' + (b - k*b1))
    badj = b - k * b1
    cols = np.stack([k, badj, s, b0, cm, slope, s * cm,
                     np.zeros(C, np.float32)], axis=1)
    pp = np.concatenate([cols, cols], axis=0).astype(np.float32)  # [128, 8]

    sw = np.sign(np.asarray(conv_w, np.float32)).astype(np.float32)

    def blockdiag(kh, kw):
        S = sw[:, :, kh, kw].T  # [ci, co]
        out = np.zeros((P, P), np.float32)
        out[0:C, 0:C] = S
        out[C:P, C:P] = S
        return out

    wp = np.zeros((P, 9, 128), np.float32)
    for kw in range(3):            # [kw, delta(kh 0/1), m] DoubleRow pairs
        for d in range(2):
            wp[:, kw * 2 + d, :] = blockdiag(d, kw)
    for d in range(2):             # kh=2: [delta(kw 0/1), m] DoubleRow
        wp[:, 6 + d, :] = blockdiag(2, d)
    wp[:, 8, :] = blockdiag(2, 2)  # kh=2, kw=2 plain
    wp = np.ascontiguousarray(wp.reshape(P, 9 * 128)).astype(
        mybir.dt.np(FP8))

    xr = np.asarray(x, np.float32) + b1[None, :, None, None]
    xr = xr.astype(ml_dtypes.bfloat16)
    in_maps = []
    for c in range(NCORES):
        xc = np.ascontiguousarray(xr[2 * c:2 * c + 2]).reshape(P, H * W)
        in_maps.append({"xin": xc, "wp": wp, "pp": pp})
    return in_maps


def kernel(x, rd_k, rd_b, beta, conv_w, pr_bias0, prelu_w, pr_bias1):
    global FAST_POST
    slope = np.asarray(prelu_w, np.float32).reshape(C)
    b0 = np.asarray(pr_bias0, np.float32).reshape(C)
    if not (np.all(b0 == 0.0) and np.all((slope >= 0.0) & (slope <= 1.0))):
        FAST_POST = False    # m-path needs b0==0; stt-max needs slope in [0,1]
    in_maps = make_inputs(x, rd_k, rd_b, beta, conv_w, pr_bias0, prelu_w,
                          pr_bias1)
    nc = _get_nc()
    res = run_bass_kernel_spmd(nc, in_maps, core_ids=list(range(NCORES)))
    y = np.empty((B, C, H, W), np.float32)
    for c in range(NCORES):
        y[2 * c:2 * c + 2] = np.asarray(
            res.results[c]["yout"]).astype(np.float32).reshape(2, C, H, W)
    return y
